# revision 49
# baseline (speedup 1.0000x reference)
"""Single-head attention kernel for Trainium2, SPMD over 8 NeuronCores.

Problem: x [4,4096,128], Wq/Wk/Wv [128,128] -> y [4,4096,128]
  q = x @ Wq.T ; k = x @ Wk.T ; v = x @ Wv.T
  y = softmax(q k^T / sqrt(128)) v

Sharding: 8 cores = 4 batches x 2 query-halves. Each core receives its
batch's x rotated so that its 2048 queries are rows 0..2047 (attention is
invariant to permuting the key order, so rotation changes nothing) -> all
cores run the identical NEFF with no dynamic offsets and no collectives.

Per-core dataflow (all attention matmuls bf16 inputs, f32 PSUM accum):
  xT chunks: load f32, cast on DVE; chunk 0 transposed on PE, chunks 1..7
  via the DMA xbar transpose engine (keeps the PE for real matmuls)
  M = Wq^T @ Wk (no transposes needed); uT = M^T @ xT[:2048]; Wv folded in
  at the end: y = (A @ x) @ Wv^T
  per 1024-query block, per 32 key tiles:
      S^T = xT-tile^T @ uT-block     (PE, 2x N=512 into [128k,1024] PSUM)
      A^T = exp(S^T * scale)         (ACT, one op per 1024, bf16 SBUF;
                                      1/4 of tiles via DVE Schraudolph)
      yT += x-tile^T @ A^T           (PE, [128h,1024q] PSUM accum)
      full bf16 pair-tree of A^T to ONE root (DVE + some GpSimd)
  l-col = root-slice^T @ ones (8 tiny PE matmuls -> [128q,1] direct, no
  row-transposes), reciprocal once per block, y-slice = (w_sb^T @ wvT) *
  (1/l) with the scale applied on ACT.

The emission is a single software-pipelined instruction stream: the PE
executes strictly in program order, so prep chunks 2..7 and each block's
epilogue are sliced into small pieces and interleaved into the next
block's kt loop.  ~8 dummy matmuls at kernel entry warm the PE's HAM
clock gate during the initial DMA wait so the real stream runs at full
clock from its first instruction.
"""

import sys

sys.path.insert(0, "/opt/trn_rl_repo")

import numpy as np

import concourse.bass as bass
import concourse.mybir as mybir
from concourse import bacc
from concourse.bass_utils import run_bass_kernel_spmd
from concourse.tile import TileContext
from concourse.masks import make_identity

P = 128
N = 4096  # context length (per batch)
NQ = 2048  # queries per core
H = 128
O = 128
KT = N // P  # 32 key tiles
NC = N // 512  # 8 column chunks of 512
QBS = 1024  # query block size
QB = NQ // QBS  # 2 query blocks
SCALE = 1.0 / np.sqrt(128.0)
# Schraudolph exp-as-bf16-bits: bf16bits(exp(s*SCALE)) ~= s*ES0 + ES1
ES0 = float(128.0 * np.log2(np.e) * SCALE)
ES1 = float((127 << 7) - 8.0)

F32 = mybir.dt.float32
BF16 = mybir.dt.bfloat16

# chunks transposed on the PE (latency-critical prologue); the rest go
# through the DMA xbar transpose engine
PE_T_CHUNKS = 1

_cached_nc = None


def build_kernel():
    nc = bacc.Bacc(None, target_bir_lowering=False)

    x_d = nc.declare_dram_parameter("x", [N, H], F32, isOutput=False)
    w_d = {
        "q": nc.declare_dram_parameter("wq", [H, H], F32, isOutput=False),
        "k": nc.declare_dram_parameter("wk", [H, H], F32, isOutput=False),
        "v": nc.declare_dram_parameter("wv", [O, H], F32, isOutput=False),
    }
    out_d = nc.declare_dram_parameter("out", [NQ, O], F32, isOutput=True)

    with TileContext(nc) as tc:
        with (
            tc.tile_pool(name="const", bufs=1) as cpool,
            tc.tile_pool(name="big", bufs=1) as big,
            tc.tile_pool(name="stagea", bufs=5) as sta,
            tc.tile_pool(name="psum", bufs=2, space="PSUM") as psum,
            tc.tile_pool(name="apool", bufs=8) as apool,
            tc.tile_pool(name="tpool", bufs=6) as tpool,
            tc.tile_pool(name="epi", bufs=3) as epi,
        ):
            xTs = [big.tile([P, 512], BF16, name=f"xT{c}") for c in range(NC)]
            uTs = [big.tile([P, 512], BF16, name=f"uT{c}") for c in range(NQ // 512)]
            xcs = [big.tile([P, 512], BF16, name=f"xc{c}") for c in range(NC)]
            wq_n = big.tile([P, P], BF16, name="wq_n")
            wk_n = big.tile([P, P], BF16, name="wk_n")
            wvT = big.tile([P, P], BF16, name="wvT")
            m_sb = big.tile([P, P], BF16, name="m_sb")

            def kslice(tiles, kt):
                return tiles[kt // 4][:, (kt % 4) * P : (kt % 4 + 1) * P]

            # ---------- prep piece emitters ----------
            def emit_ham_warm():
                # dummy matmuls during the initial DMA wait: keeps the PE
                # busy so the HAM clock-gate opens (1.2 -> 2.4 GHz) before
                # the real stream starts.  The garbage operand only needs a
                # gpsimd memset, which completes well before the first DMA.
                g = cpool.tile([P, 512], BF16, name="warmg")
                nc.gpsimd.memset(g[:], 1.0)
                pw = psum.tile([P, 512], F32, tag="sm")
                for _ in range(15):
                    nc.tensor.matmul(
                        pw[:], g[:, 0:P], g[:], start=True, stop=True
                    )

            def emit_w_setup_loads():
                # one 3-W f32 load on the sync HWDGE queue (issued after the
                # first x chunks), casts on ACT which is idle pre-exp; avoids
                # the ~3.5us SWDGE cast-DMA completion latency on the M chain
                wf = sta.tile([P, 3 * P], F32, tag="wf")
                for wi, name in enumerate(("q", "k", "v")):
                    nc.scalar.dma_start(
                        out=wf[:, wi * P : (wi + 1) * P], in_=w_d[name][:]
                    )
                wst = sta.tile([P, P], BF16, tag="wst")
                for wi, wt in enumerate((wq_n, wk_n, wst)):
                    nc.scalar.activation(
                        wt[:], wf[:, wi * P : (wi + 1) * P],
                        mybir.ActivationFunctionType.Copy,
                    )
                ident_bf_ = cpool.tile([P, P], BF16, name="ident_bf")
                make_identity(nc, ident_bf_)
                ones_bf_ = cpool.tile([P, 1], BF16, name="ones_bf")
                nc.gpsimd.memset(ones_bf_[:], 1.0)
                return ident_bf_, ones_bf_, wst

            def emit_w_setup_pe(wst):
                pm = psum.tile([P, P], F32, tag="sm")
                nc.tensor.matmul(pm[:], wq_n[:], wk_n[:], start=True, stop=True)
                nc.vector.tensor_copy(m_sb[:], pm[:])
                pw = psum.tile([P, 2 * P], BF16, tag="sm")
                nc.tensor.transpose(pw[:, 0:P], wst[:], ident_bf[:])
                nc.vector.tensor_copy(wvT[:], pw[:, 0:P])

            chunk_state = {}

            xsts = {}

            def chunk_dma(c, eng=None):
                # loads split across both hwdge rings; bufs=8 so no load
                # ever WAR-waits at the queue head (which would block every
                # later DMA behind it)
                xst = sta.tile([P, 4, P], F32, tag="xst", bufs=8, name=f"xst{c}")
                xsts[c] = xst
                rows = x_d[c * 512 : (c + 1) * 512, :]
                (eng or nc.sync).dma_start(
                    out=xst[:], in_=rows.rearrange("(t p) h -> p t h", p=P)
                )

            def chunk_cast(c):
                nc.vector.tensor_copy(
                    xcs[c][:], xsts[c].rearrange("p t h -> p (t h)")
                )  # f32 -> bf16

            def chunk_transpose_pe(c, half=None):
                if half in (None, 0):
                    px = psum.tile([P, 512], BF16, tag="sm", name=f"px{c}")
                    chunk_state[(c, "px")] = px
                px = chunk_state[(c, "px")]
                rng = range(4) if half is None else range(half * 2, half * 2 + 2)
                for t4 in rng:
                    nc.tensor.transpose(
                        px[:, t4 * P : (t4 + 1) * P],
                        xcs[c][:, t4 * P : (t4 + 1) * P],
                        ident_bf[:],
                    )
                if half in (None, 1):
                    nc.vector.tensor_copy(xTs[c][:], px[:])
                    del chunk_state[(c, "px")]

            def chunk_transpose_dma(c):
                # whole-chunk SBUF->SBUF transpose via the DMA xbar: with a
                # 3D out AP [p, t, f] the engine computes out[p,t,f] =
                # in[f, t*128+p], i.e. all four 128x128 tile transposes in
                # one instruction (one ~1.3us sync-queue issue, not four)
                nc.sync.dma_start_transpose(
                    out=xTs[c][:].rearrange("p (t f) -> p t f", t=4),
                    in_=xcs[c][:],
                )

            def chunk_u(c):
                pu = psum.tile([P, 512], F32, tag="sm", name=f"pu{c}")
                nc.tensor.matmul(pu[:], m_sb[:], xTs[c][:], start=True, stop=True)
                nc.scalar.activation(
                    uTs[c][:], pu[:], mybir.ActivationFunctionType.Copy
                )

            # ---------- attention emitters ----------
            a_tiles = {}

            def emit_s_exp(qb, kt):
                ps = psum.tile([P, QBS], F32, tag="ps")
                for h in range(QBS // 512):
                    nc.tensor.matmul(
                        ps[:, h * 512 : (h + 1) * 512],
                        kslice(xTs, kt),
                        uTs[qb * 2 + h][:],
                        start=True, stop=True,
                    )
                a = apool.tile([P, QBS], BF16, tag="a")
                if kt % 4 == 2 or kt % 16 == 7:
                    # Schraudolph: affine map + f32->int16 convert writes the
                    # bf16 bit pattern of exp(s*SCALE) (~2% elementwise, which
                    # the softmax renormalization cancels to ~0.1% on y)
                    nc.vector.tensor_scalar(
                        a.bitcast(mybir.dt.int16)[:], ps[:], ES0, ES1,
                        mybir.AluOpType.mult, mybir.AluOpType.add,
                    )
                else:
                    nc.scalar.activation(
                        a[:], ps[:], mybir.ActivationFunctionType.Exp,
                        scale=float(SCALE),
                    )
                a_tiles[(qb, kt)] = a

            def new_blk(qb):
                # single shared py buffer: the kt order is rotated so the
                # first write of block qb+1 lands after block qb's w_sb
                # copies have drained (frees 4KB of PSUM for a 3rd ps buf)
                return {
                    "qb": qb,
                    "py": psum.tile([P, QBS], F32, tag="py", bufs=1,
                                    name=f"py{qb}"),
                    "tree": {},
                    "nleaf": 0,
                    "root": None,
                }

            def tree_insert(b, cur, kt):
                # For the LAST block only: binary counter to 28 leaves, then
                # eager-merge the pending levels and add the final 4 leaves
                # serially -- same 31 adds, but only ONE depends on the last
                # leaf, so the tail's root-cascade PE-idle gap shrinks ~3us.
                # (Applying this to both blocks regressed: the serial chain
                # couples into the next block's DVE traffic.)
                serial = b["qb"] == QB - 1
                tree = b["tree"]
                b["nleaf"] += 1
                n = b["nleaf"]
                if not serial or n <= KT - 4:
                    lvl = 1
                    while tree.get(lvl) is not None:
                        prev = tree[lvl]
                        tree[lvl] = None
                        out = tpool.tile([P, QBS], BF16, tag=f"t{lvl}")
                        eng = (
                            nc.gpsimd
                            if (lvl == 1 and kt % 4 == 1 and kt < KT - 8)
                            else nc.vector
                        )
                        eng.tensor_tensor(
                            out[:], prev[:], cur[:], mybir.AluOpType.add
                        )
                        cur = out
                        lvl += 1
                    tree[lvl] = cur
                    if n == KT:
                        b["root"] = cur
                        b["tree"] = {}
                    elif serial and n == KT - 4:
                        pend = [tree[l] for l in sorted(tree)
                                if tree[l] is not None]
                        r = pend[0]
                        for p in pend[1:]:
                            out = tpool.tile([P, QBS], BF16, tag="tR")
                            nc.vector.tensor_tensor(
                                out[:], r[:], p[:], mybir.AluOpType.add
                            )
                            r = out
                        b["R"] = r
                        b["tree"] = {}
                else:
                    out = tpool.tile([P, QBS], BF16, tag="tR")
                    nc.vector.tensor_tensor(
                        out[:], b["R"][:], cur[:], mybir.AluOpType.add
                    )
                    b["R"] = out
                    if n == KT:
                        b["root"] = out

            def emit_kt(b, kt, first, final):
                qb = b["qb"]
                a = a_tiles.pop((qb, kt))
                for h in range(QBS // 512):
                    nc.tensor.matmul(
                        b["py"][:, h * 512 : (h + 1) * 512],
                        kslice(xcs, kt),
                        a[:, h * 512 : (h + 1) * 512],
                        start=first, stop=final,
                    )
                tree_insert(b, a, kt)

            def epilogue_pieces(b, last=False):
                """The block epilogue as a list of small closures."""
                qb = b["qb"]
                st = {}

                def p_w(h):
                    if h == 0:
                        st["w_sb"] = epi.tile(
                            [P, QBS], BF16, tag="w_sb", name=f"w{qb}"
                        )
                    if last:
                        # tail critical path is DVE-serialized (final tree
                        # R-adds); ACT is idle after the last exp, so copy
                        # there to unblock the psm_y matmuls ~2us sooner
                        nc.scalar.activation(
                            st["w_sb"][:, h * 512 : (h + 1) * 512],
                            b["py"][:, h * 512 : (h + 1) * 512],
                            mybir.ActivationFunctionType.Copy,
                        )
                    else:
                        nc.vector.tensor_copy(
                            st["w_sb"][:, h * 512 : (h + 1) * 512],
                            b["py"][:, h * 512 : (h + 1) * 512],
                        )

                def p_l(j2):
                    # lcol[q,1] = root[:, qslice]^T @ ones  (tiny matmuls,
                    # replaces the old ones^T@roots row + per-j transposes)
                    if j2 == 0:
                        st["pl"] = psum.tile(
                            [P, 8], F32, tag="sm", name=f"pl{qb}"
                        )
                    root = b["root"]
                    for j in range(j2 * 4, j2 * 4 + 4):
                        nc.tensor.matmul(
                            st["pl"][:, j : j + 1],
                            root[:, j * P : (j + 1) * P],
                            ones_bf[:],
                            start=True, stop=True,
                        )
                    if j2 == 1:
                        lr = epi.tile([P, 8], F32, tag="lrec", name=f"lr{qb}")
                        nc.vector.reciprocal(lr[:], st["pl"][:])
                        st["lrec"] = lr

                def p_j(g, j4):
                    j = g * 4 + j4
                    if j4 == 0:
                        st[f"yout{g}"] = epi.tile(
                            [P, 4, P], F32, tag="yout", name=f"yout{qb}_{g}"
                        )
                    psm_y = psum.tile([P, P], F32, tag="sm")
                    nc.tensor.matmul(
                        psm_y[:], st["w_sb"][:, j * P : (j + 1) * P], wvT[:],
                        start=True, stop=True,
                    )
                    nc.scalar.activation(
                        st[f"yout{g}"][:, j4, :], psm_y[:],
                        mybir.ActivationFunctionType.Copy,
                        scale=st["lrec"][:, j : j + 1],
                    )
                    if j4 == 3:
                        r0 = qb * QBS + g * 512
                        if last and g == 1:
                            # split the final store so the tail DMA is short
                            for hh in range(2):
                                nc.sync.dma_start(
                                    out=out_d[
                                        r0 + hh * 256 : r0 + (hh + 1) * 256, :
                                    ].rearrange("(t p) h -> p t h", p=P),
                                    in_=st[f"yout{g}"][:, hh * 2 : hh * 2 + 2, :],
                                )
                        else:
                            nc.sync.dma_start(
                                out=out_d[r0 : r0 + 512, :].rearrange(
                                    "(t p) h -> p t h", p=P
                                ),
                                in_=st[f"yout{g}"][:],
                            )

                if last:
                    pieces = [lambda: p_w(0), lambda: p_w(1),
                              lambda: p_l(0), lambda: p_l(1)]
                else:
                    # slots 0-3 empty: the root's final tree adds are still
                    # cascading on the DVE (p_l would head-of-line stall
                    # the PE) and the boundary Schraudolph needs the DVE
                    # (so the w_sb copies move to slots 6-7)
                    pieces = [lambda: None, lambda: None,
                              lambda: None, lambda: None,
                              lambda: p_l(0), lambda: p_l(1),
                              lambda: p_w(0), lambda: p_w(1)]
                for g in range(2):
                    for j4 in range(4):
                        pieces.append(lambda g=g, j4=j4: p_j(g, j4))
                return pieces

            # ---------- emission schedule ----------
            DEPTH = 2
            emit_ham_warm()
            chunk_dma(0, nc.scalar)
            ident_bf, ones_bf, _wst = emit_w_setup_loads()
            chunk_dma(1, nc.scalar)
            chunk_dma(2, nc.sync)
            chunk_dma(3, nc.sync)
            chunk_dma(4, nc.sync)
            chunk_dma(5, nc.sync)
            chunk_cast(0)
            chunk_cast(1)
            emit_w_setup_pe(_wst)
            chunk_transpose_pe(0)
            chunk_u(0)
            chunk_transpose_pe(1)
            chunk_u(1)
            # filler dummies: bridge the PE-idle window between the prep
            # matmuls and the first S^T (inputs not ready yet) so the HAM
            # clock-gate never sees an idle window and re-throttles
            warm_g = cpool.tile([P, 512], BF16, name="warmg2")
            nc.gpsimd.memset(warm_g[:], 1.0)
            pw2 = psum.tile([P, 512], F32, tag="sm")
            for _ in range(4):
                nc.tensor.matmul(
                    pw2[:], warm_g[:, 0:P], warm_g[:], start=True, stop=True
                )
            ORDER = list(range(KT))
            blk = None
            for qb in range(QB):
                if qb == 0:
                    for j in range(DEPTH):
                        emit_s_exp(0, ORDER[j])
                nxt = new_blk(qb)
                if blk is None:
                    # prep pieces for chunks 2..7, spread over kts
                    todo = []
                    for c in range(2, 4):
                        # chunks 2-3 are needed early (kt8/kt12) while the
                        # PE is still input-starved: transpose them on the
                        # PE (fills idle, keeps the HAM clock-gate open)
                        todo.append(lambda c=c: chunk_cast(c))
                        todo.append(lambda c=c: chunk_transpose_pe(c, 0))
                        todo.append(lambda c=c: chunk_transpose_pe(c, 1))
                        todo.append(lambda c=c: chunk_u(c))
                    for c in range(4, NC):
                        todo.append(lambda c=c: chunk_cast(c))

                        def t_and_load(c=c):
                            # xbar transpose of chunk c, then the NEXT load
                            # on the same sync ring
                            chunk_transpose_dma(c)
                            if c + 2 < NC:
                                chunk_dma(c + 2, nc.sync)

                        todo.append(t_and_load)
                else:
                    todo = epilogue_pieces(blk)
                blk = nxt

                for i, kt in enumerate(ORDER):
                    if i + DEPTH < KT:
                        emit_s_exp(qb, ORDER[i + DEPTH])
                    elif qb + 1 < QB:
                        emit_s_exp(qb + 1, ORDER[i + DEPTH - KT])
                    emit_kt(blk, kt, first=(i == 0), final=(i == KT - 1))
                    if todo:
                        todo.pop(0)()
                while todo:
                    todo.pop(0)()
            for piece in epilogue_pieces(blk, last=True):
                piece()

    nc.compile()
    return nc


def _run(x, Wq, Wk, Wv, **spmd_kwargs):
    global _cached_nc
    if _cached_nc is None:
        _cached_nc = build_kernel()
    nc = _cached_nc

    x = np.asarray(x, dtype=np.float32)
    Wq = np.ascontiguousarray(np.asarray(Wq, dtype=np.float32))
    Wk = np.ascontiguousarray(np.asarray(Wk, dtype=np.float32))
    Wv = np.ascontiguousarray(np.asarray(Wv, dtype=np.float32))

    B = x.shape[0]
    in_maps = []
    for core in range(8):
        b, half = core // 2, core % 2
        xb = x[b]
        if half:
            xb = np.roll(xb, -NQ, axis=0)  # queries -> rows 0..NQ-1
        in_maps.append(
            {"x": np.ascontiguousarray(xb), "wq": Wq, "wk": Wk, "wv": Wv}
        )

    res = run_bass_kernel_spmd(nc, in_maps, core_ids=list(range(8)), **spmd_kwargs)

    y = np.empty((B, N, O), dtype=np.float32)
    for core in range(8):
        b, half = core // 2, core % 2
        y[b, half * NQ : (half + 1) * NQ] = res.results[core]["out"]
    return y, res


def kernel(x, Wq, Wk, Wv):
    y, _ = _run(x, Wq, Wk, Wv)
    return y


if __name__ == "__main__":
    rng = np.random.default_rng(0)
    x = rng.standard_normal((4, N, H), dtype=np.float32)
    Wq = rng.standard_normal((H, H), dtype=np.float32) / np.sqrt(H)
    Wk = rng.standard_normal((H, H), dtype=np.float32) / np.sqrt(H)
    Wv = rng.standard_normal((H, H), dtype=np.float32) / np.sqrt(H)
    y = kernel(x=x, Wq=Wq, Wk=Wk, Wv=Wv)
    print("kernel output", y.shape, y.dtype)
